# revision 30
# baseline (speedup 1.0000x reference)
"""Trainium2 Bass kernel for nn_Decoder2 (dense transformer decoder block).

Sharding (8 cores):
  - both attentions: head-sharded, 2 heads (=128 feature dims) per core
  - FFN: hidden dim column/row sharded, 512 hidden units per core; the 8
    partial outputs are summed on the host
  - wemb/pemb replicated; all activations kept transposed [feat, seq]

v2 structure (vs v1):
  - everything bf16: weights host-prepacked bf16, activations written bf16
    out of PSUM, output partials stored bf16 (host sums in f64)
  - W2 fully resident in SBUF (no per-step weight DMA in the FFN tail)
  - causal mask applied as an additive PE matmul (identity x extended-mask
    view) accumulated into the scores PSUM group - nothing on the DVE chain
  - per-chunk AllGathers (8 x [128,512]) issued right after each chunk's
    normalize, so collectives pipeline against the next chunk's attention
  - chunk c+1's q/k/v projections run as fillers inside chunk c's
    attention loop (no serial projection phases)

Softmax is computed without max-subtraction (scores are O(+-6)); the
softmax denominator comes from a ones-column folded into the AV matmul
(lhsT = [v_head | ones], m=65). Scores for the two heads are issued
adjacently as K=64 row-tiles (tile_position) so they run concurrently.
"""

import ml_dtypes
import numpy as np

import concourse.bass as bass
import concourse.bacc as bacc
import concourse.mybir as mybir
import concourse.tile as tile
from concourse.bass_utils import run_bass_kernel_spmd
from concourse.masks import make_identity

F32 = mybir.dt.float32
BF16 = mybir.dt.bfloat16
AF = mybir.ActivationFunctionType

N_CORES = 8
S_W, S_P = 2048, 1024
D_MODEL, NEW_DIM, H, D_FF = 1024, 1024, 16, 4096
HD = 128          # head-feature dims per core (2 heads x 64)
FF_SH = D_FF // N_CORES   # 512 hidden units per core
NC = 512          # free-dim chunk for matmuls
DCH = D_MODEL // 128      # 8 contraction chunks of 128
NSQ = S_W // NC           # 4 sq chunks
NSKB = S_W // 128         # 16 self key blocks
NSPB = S_P // 128         # 8 cross key blocks
NFB = FF_SH // 128        # 4 ffn hidden blocks per core
MASKV = -1.0e5            # additive causal-mask value (pre exp-scale 0.125)


def decoder_kernel(tc):
    nc = tc.nc

    # all inputs host-prepacked bf16, [128, ...] partition-major contiguous
    wembT = nc.dram_tensor("wembT", [128, NSQ * DCH * NC], BF16,
                           kind="ExternalInput").ap()
    pembT = nc.dram_tensor("pembT", [128, 2 * DCH * NC], BF16,
                           kind="ExternalInput").ap()
    wqmT = nc.dram_tensor("wqmT", [128, DCH * HD], BF16, kind="ExternalInput").ap()
    wkmT = nc.dram_tensor("wkmT", [128, DCH * HD], BF16, kind="ExternalInput").ap()
    wvmT = nc.dram_tensor("wvmT", [128, DCH * HD], BF16, kind="ExternalInput").ap()
    wqcT = nc.dram_tensor("wqcT", [128, DCH * HD], BF16, kind="ExternalInput").ap()
    wkcT = nc.dram_tensor("wkcT", [128, DCH * HD], BF16, kind="ExternalInput").ap()
    wvcT = nc.dram_tensor("wvcT", [128, DCH * HD], BF16, kind="ExternalInput").ap()
    w1T = nc.dram_tensor("w1T", [128, DCH * FF_SH], BF16, kind="ExternalInput").ap()
    w2T = nc.dram_tensor("w2T", [128, DCH * NFB * 128], BF16,
                         kind="ExternalInput").ap()
    outT = nc.dram_tensor("outT", [D_MODEL, S_W], BF16, kind="ExternalOutput").ap()

    rg = [list(range(N_CORES))]

    with (
        tc.tile_pool(name="const", bufs=1) as constp,
        tc.tile_pool(name="dram", bufs=1, space="DRAM") as dramp,
        tc.tile_pool(name="big", bufs=1) as bigp,
        tc.tile_pool(name="chunk", bufs=2) as chkp,
        tc.tile_pool(name="work", bufs=2) as workp,
        tc.tile_pool(name="ps_pp", bufs=2, space="PSUM") as ps_pp,
        tc.tile_pool(name="ps_s", bufs=2, space="PSUM") as ps_s,
        tc.tile_pool(name="ps_o", bufs=1, space="PSUM") as ps_o,
    ):
        # ---- big persistent tiles ----
        qT = bigp.tile([128, S_W], BF16, tag="qT", name="qT")
        kT = bigp.tile([128, S_W], BF16, tag="kT", name="kT")
        v65 = bigp.tile([128, NSKB * 130], BF16, tag="v65", name="v65")
        kcT = bigp.tile([128, S_P], BF16, tag="kcT", name="kcT")
        vc65 = bigp.tile([128, NSPB * 130], BF16, tag="vc65", name="vc65")

        # ---- weight DMAs (scalar queue; wemb chunk 0 goes on sync queue) ----
        def load_wT(dram_ap, tag, name):
            t = constp.tile([128, DCH * HD], BF16, tag=tag, name=name)
            nc.scalar.dma_start(t[:], dram_ap)
            return t

        wq_sb = load_wT(wqmT, "wq", "wqm")
        wk_sb = load_wT(wkmT, "wk", "wkm")
        wv_sb = load_wT(wvmT, "wv", "wvm")

        # ---- x chunk loads (sync queue, 4-way split) ----
        def xcat_load(dram_ap, name):
            t = chkp.tile([128, DCH * NC], BF16, tag="xcat", name=name)
            q = DCH * NC // 4
            for i in range(4):
                nc.sync.dma_start(t[:, q * i:q * (i + 1)],
                                  dram_ap[:, q * i:q * (i + 1)])
            return [t[:, NC * dc:NC * (dc + 1)] for dc in range(DCH)]

        def dma_wemb(c):
            return xcat_load(
                wembT[:, DCH * NC * c:DCH * NC * (c + 1)], f"wemb_{c}")

        def pemb_chunks(sc):
            return xcat_load(
                pembT[:, DCH * NC * sc:DCH * NC * (sc + 1)], f"pemb_{sc}")

        # issue chunk-0 x load before constants so the wire time is hidden
        xw = {0: dma_wemb(0)}

        # ---- constants ----
        identB = constp.tile([128, 128], BF16, tag="ident")
        make_identity(nc, identB[:])
        # additive causal mask: emask[p, yy] = 0 iff yy - p >= 896 else MASKV.
        # view(k) = emask[:, 896-128k : 1408-128k] gives, for query col x and
        # key row p: 0 iff x >= 128k + p (valid), else MASKV.  k in 0..3 is
        # the diagonal sub-block offset; larger views would be fully masked.
        emask = constp.tile([128, 1408], BF16, tag="emask")
        nc.gpsimd.memset(emask[:], 0.0)
        nc.gpsimd.affine_select(
            out=emask[:], in_=emask[:],
            compare_op=mybir.AluOpType.is_ge,
            fill=MASKV,
            base=-896,
            pattern=[[1, 1408]],
            channel_multiplier=-1,
        )

        # softmax-denominator ones columns (written once; rows 64/129 of each
        # 130-wide v block stay 1.0 for the whole kernel)
        for b in range(NSKB):
            nc.vector.memset(v65[:, 130 * b + 64:130 * b + 65], 1.0)
            nc.vector.memset(v65[:, 130 * b + 129:130 * b + 130], 1.0)
        for b in range(NSPB):
            nc.vector.memset(vc65[:, 130 * b + 64:130 * b + 65], 1.0)
            nc.vector.memset(vc65[:, 130 * b + 129:130 * b + 130], 1.0)

        wqc_sb = load_wT(wqcT, "wq2", "wqc")
        wkc_sb = load_wT(wkcT, "wk2", "wkc")
        wvc_sb = load_wT(wvcT, "wv2", "wvc")

        # warm the ACT exp spline table off the critical path
        warm_in = constp.tile([1, 8], F32, tag="warm")
        nc.vector.memset(warm_in[:], 0.0)
        warm_out = constp.tile([1, 8], F32, tag="warm2")
        nc.scalar.activation(warm_out[:], warm_in[:], AF.Exp, scale=0.125)

        # FFN weights fully resident
        w1_sb = constp.tile([128, DCH * FF_SH], BF16, tag="w1", name="w1")
        w2_sb = constp.tile([128, DCH * NFB * 128], BF16, tag="w2", name="w2")
        for i in range(4):
            q = DCH * FF_SH // 4
            nc.gpsimd.dma_start(w1_sb[:, q * i:q * (i + 1)],
                                w1T[:, q * i:q * (i + 1)])
            nc.gpsimd.dma_start(w2_sb[:, q * i:q * (i + 1)],
                                w2T[:, q * i:q * (i + 1)])

        # ---- projection helpers ----
        def proj_chunk(out_ap, w_sb, x_chunks):
            ps = ps_pp.tile([128, NC], F32, tag="pp", name="ps_pj")
            for dc in range(DCH):
                nc.tensor.matmul(
                    ps[:],
                    w_sb[:, HD * dc:HD * (dc + 1)],
                    x_chunks[dc][:],
                    start=(dc == 0),
                    stop=(dc == DCH - 1),
                )
            nc.vector.tensor_copy(out_ap, ps[:])

        def transp_block(v65_sb, vt_c, lb, b):
            ps = ps_pp.tile([128, 128], BF16, tag="pp", name="ps_tr")
            nc.tensor.transpose(ps[:], vt_c[:, 128 * lb:128 * (lb + 1)],
                                identB[:])
            nc.vector.tensor_copy(v65_sb[:, 130 * b:130 * b + 64], ps[:, 0:64])
            nc.vector.tensor_copy(
                v65_sb[:, 130 * b + 65:130 * b + 129], ps[:, 64:128])

        def proj_q(c, xc):
            proj_chunk(qT[:, NC * c:NC * (c + 1)], wq_sb, xc)

        def proj_k(c, xc):
            proj_chunk(kT[:, NC * c:NC * (c + 1)], wk_sb, xc)

        def proj_v(c, xc):
            vtc = chkp.tile([128, NC], BF16, tag="vt", name=f"vT{c}", bufs=3)
            proj_chunk(vtc[:], wv_sb, xc)
            for lb in range(4):
                transp_block(v65, vtc, lb, 4 * c + lb)

        def proj_kc(sc, xc):
            proj_chunk(kcT[:, NC * sc:NC * (sc + 1)], wkc_sb, xc)

        def proj_vc(sc, xc):
            vtc = chkp.tile([128, NC], BF16, tag="vt", name=f"vcT{sc}", bufs=3)
            proj_chunk(vtc[:], wvc_sb, xc)
            for lb in range(4):
                transp_block(vc65, vtc, lb, 4 * sc + lb)

        # ---- attention chunk ----
        # Per j-step: both heads' scores go into one [128,1024] PSUM pair
        # (adjacent K=64 row-tiles, concurrent); on diagonal blocks the
        # additive mask matmul joins the accumulation group; ONE exp over
        # both heads writes bf16; then two m=65 AV matmuls (ones-column ->
        # softmax denominator in row 64).  `fillers` emits one unit of
        # independent PE work after each j-step.
        def attention_chunk(out_c, q_ap, k_sb, v65_sb, n_j, causal_c,
                            fillers=()):
            fill = iter(fillers)
            pso = [ps_o.tile([65, NC], F32, tag=f"o{h}", name=f"pso{h}")
                   for h in range(2)]
            for j in range(n_j):
                pss = ps_s.tile([128, 2 * NC], F32, tag="s", name="pss")
                masked = causal_c is not None and j >= 4 * causal_c
                for h in range(2):
                    nc.tensor.matmul(
                        pss[:, NC * h:NC * (h + 1)],
                        k_sb[64 * h:64 * (h + 1), 128 * j:128 * (j + 1)],
                        q_ap[64 * h:64 * (h + 1), :],
                        start=True, stop=not masked,
                        tile_position=(64 * h, 0),
                    )
                if masked:
                    off = 896 - 128 * (j - 4 * causal_c)
                    for h in range(2):
                        nc.tensor.matmul(
                            pss[:, NC * h:NC * (h + 1)],
                            identB[:],
                            emask[:, off:off + NC],
                            start=False, stop=True,
                        )
                es = workp.tile([128, 2 * NC], BF16, tag="e", name="es")
                nc.scalar.activation(es[:], pss[:], AF.Exp, scale=0.125)
                for h in range(2):
                    nc.tensor.matmul(
                        pso[h][:],
                        v65_sb[:, 130 * j + 65 * h:130 * j + 65 * h + 65],
                        es[:, NC * h:NC * (h + 1)],
                        start=(j == 0),
                        stop=(j == n_j - 1),
                    )
                th = next(fill, None)
                if th is not None:
                    th()
            for th in fill:
                if th is not None:
                    th()
            for h in range(2):
                lrow = workp.tile([1, NC], F32, tag="lrow", name="lrow")
                nc.vector.tensor_copy(lrow[:], pso[h][64:65, :])
                rec = workp.tile([1, NC], F32, tag="rec", name="rec")
                nc.vector.reciprocal_approx_fast(rec[:], lrow[:])
                rec64 = workp.tile([64, NC], F32, tag="rec64", name="rec64")
                nc.gpsimd.partition_broadcast(rec64[:], rec[:])
                nc.vector.tensor_mul(
                    out_c[64 * h:64 * (h + 1), :], pso[h][0:64, :], rec64[:])

        # ---- collectives ----
        # Word AGs are paired (two [128,1024] ops): the CC runtime's ~72us
        # init gates the first op anyway, and fewer ops unblock the cross-AG
        # train sooner.  Cross AGs stay per-chunk for tail pipelining.
        def allgather(src_sb, name, width):
            bounce = dramp.tile([128, width], BF16, name=f"bnc_{name}")
            gath = dramp.tile([N_CORES * 128, width], BF16, name=f"gd_{name}",
                              addr_space="Shared")
            nc.gpsimd.dma_start(bounce[:], src_sb[:])
            nc.gpsimd.collective_compute(
                "AllGather",
                mybir.AluOpType.bypass,
                replica_groups=rg,
                ins=[bounce[:].opt()],
                outs=[gath[:].opt()],
            )
            return gath

        wd_c = {}
        cd_c = {}
        qc_t = {}

        def gather_read(t, g):
            # one 3D-pattern DMA instead of 8 per-block triggers: the
            # AG->consumer edge is on the collective-train critical path
            gv = g[:, :].rearrange("(dc p) x -> p dc x", dc=DCH)
            tv = t[:].rearrange("p (dc x) -> p dc x", dc=DCH)
            nc.sync.dma_start(tv, gv)

        def qc_proj(c):
            t = chkp.tile([128, DCH * NC], BF16, tag="wdcat",
                          name=f"word_{c}", bufs=2)
            gather_read(t, wd_c[c])
            xq = [t[:, NC * dc:NC * (dc + 1)] for dc in range(DCH)]
            qc = chkp.tile([128, NC], BF16, tag=f"qc{c % 2}", name=f"qcT{c}")
            proj_chunk(qc[:], wqc_sb, xq)
            qc_t[c] = qc

        # ---- FFN ----
        ffn_state = {}

        def ffn_load(c):
            t = chkp.tile([128, DCH * NC], BF16, tag="xcat", name=f"cr_{c}")
            gather_read(t, cd_c[c])
            xc = [t[:, NC * dc:NC * (dc + 1)] for dc in range(DCH)]
            ffn_state[c] = (xc, [])

        def ffn1(c, fb):
            xc, hts = ffn_state[c]
            ps = ps_pp.tile([128, NC], F32, tag="pp", name="ps_f1")
            for dc in range(DCH):
                nc.tensor.matmul(
                    ps[:],
                    w1_sb[:, FF_SH * dc + 128 * fb:FF_SH * dc + 128 * (fb + 1)],
                    xc[dc][:],
                    start=(dc == 0),
                    stop=(dc == DCH - 1),
                )
            ht = chkp.tile([128, NC], BF16, tag=f"h{fb}", name=f"hT{fb}_{c}",
                           bufs=2)
            # relu on the scalar engine: the DVE is the tail bottleneck
            nc.scalar.activation(ht[:], ps[:], AF.Relu)
            hts.append(ht)

        def ffn2(c, ob):
            hts = ffn_state[c][1]
            ps = ps_pp.tile([128, NC], F32, tag="pp", name="ps_f2")
            for fc in range(NFB):
                nc.tensor.matmul(
                    ps[:],
                    w2_sb[:, (NFB * ob + fc) * 128:(NFB * ob + fc + 1) * 128],
                    hts[fc][:],
                    start=(fc == 0),
                    stop=(fc == NFB - 1),
                )
            o_sb = workp.tile([128, NC], BF16, tag="o_sb", name="o_sb")
            # alternate engines so PSUM drain doesn't serialize on the DVE
            if ob % 2 == 0:
                nc.scalar.copy(o_sb[:], ps[:])
            else:
                nc.vector.tensor_copy(o_sb[:], ps[:])
            nc.gpsimd.dma_start(
                outT[128 * ob:128 * (ob + 1), NC * c:NC * (c + 1)], o_sb[:])

        def ffn_thunks(c):
            ts = [lambda c=c: ffn_load(c)]
            ts += [lambda c=c, fb=fb: ffn1(c, fb) for fb in range(NFB)]
            ts += [lambda c=c, ob=ob: ffn2(c, ob) for ob in range(DCH)]
            return ts

        # Dummy matmuls that keep the PE HAM-warm through exp-bound
        # attention stretches with no real filler work left: an idle window
        # drops the PE clock gate to 1.2GHz and everything after pays 2x.
        # They write the projection-pool PSUM (never the live scores pool).
        def pe_warm(n):
            for i in range(n):
                ps = ps_pp.tile([128, NC], F32, tag="pp", name="warm")
                nc.tensor.matmul(ps[:], identB[:], kT[:, 0:NC],
                                 start=True, stop=True)

        # ---- the pipeline ----
        xp = {}

        def f_dma_w(c):
            xw[c] = dma_wemb(c)

        def f_dma_p(sc):
            xp[sc] = pemb_chunks(sc)

        def self_fillers(c):
            if c == 0:
                return [lambda: f_dma_w(1),
                        lambda: proj_q(1, xw[1]),
                        lambda: proj_k(1, xw[1]),
                        lambda: proj_v(1, xw[1])]
            if c == 1:
                return [lambda: f_dma_w(2),
                        lambda: proj_q(2, xw[2]),
                        lambda: proj_k(2, xw[2]),
                        lambda: proj_v(2, xw[2]),
                        lambda: f_dma_p(0),
                        lambda: proj_kc(0, xp[0])]
            if c == 2:
                return [lambda: f_dma_w(3),
                        lambda: proj_q(3, xw[3]),
                        lambda: proj_k(3, xw[3]),
                        lambda: proj_v(3, xw[3]),
                        lambda: proj_vc(0, xp[0]),
                        lambda: f_dma_p(1),
                        lambda: proj_kc(1, xp[1])]
            # qc0 drains after the j-loop, right as AG_w0 lands; warm units
            # keep the PE clock up through this exp-bound stretch
            return ([lambda: proj_vc(1, xp[1])]
                    + [lambda: pe_warm(1)] * 14
                    + [lambda: qc_proj(0)])

        # chunk 0 projections inline (nothing to overlap them with yet)
        proj_q(0, xw[0])
        proj_k(0, xw[0])
        proj_v(0, xw[0])

        for c in range(NSQ):
            so_c = chkp.tile([128, NC], BF16, tag=f"oa{c % 2}",
                             name=f"self_{c}")
            attention_chunk(so_c[:], qT[:, NC * c:NC * (c + 1)], kT, v65,
                            4 * (c + 1), causal_c=c, fillers=self_fillers(c))
            wd_c[c] = allgather(so_c, f"w{c}", NC)

        # qc_proj(c) consumes AG_w{c}; the AG train is serialized on the CC
        # stream behind its ~80us init, so each qc sits one cross chunk
        # behind its AG.  ffn loads are pure-DMA units (they wait on the
        # sync queue, not the PE), so they issue a chunk early.
        def cross_fillers(c):
            pw = lambda: pe_warm(2)  # noqa: E731
            if c == 0:
                return [pw, pw, pw, pw, lambda: qc_proj(1), pw, pw, pw]
            if c == 1:
                return [pw, pw, pw, pw, lambda: qc_proj(2), pw, pw, pw]
            if c == 2:
                return [pw, pw, pw, pw, lambda: qc_proj(3), pw, pw, pw]
            return [pw, lambda: ffn_load(0), pw, pw, pw, pw, pw, pw]

        for c in range(NSQ):
            co_c = chkp.tile([128, NC], BF16, tag=f"oc{c % 2}",
                             name=f"cross_{c}")
            attention_chunk(co_c[:], qc_t[c][:], kcT, vc65, NSPB,
                            causal_c=None, fillers=cross_fillers(c))
            cd_c[c] = allgather(co_c, f"c{c}", NC)

        # Tail warm-keeper across the AG_c3 stall: the scores pool is free
        # once attention is done, so these use ps_s instead of ps_pp (which
        # the ffn stages still rotate through).
        def pe_warm_tail(n):
            for i in range(n):
                ps = ps_s.tile([128, NC], F32, tag="s", name="warm")
                nc.tensor.matmul(ps[:], identB[:], kT[:, 0:NC],
                                 start=True, stop=True)

        # tail: each chunk's gather-read DMA issues one chunk ahead of its
        # ffn1 batch so the sync queue prefetches while the PE streams
        for th in ([lambda fb=fb: ffn1(0, fb) for fb in range(NFB)]
                   + [lambda: ffn_load(1)]
                   + [lambda ob=ob: ffn2(0, ob) for ob in range(DCH)]
                   + [lambda fb=fb: ffn1(1, fb) for fb in range(NFB)]
                   + [lambda: ffn_load(2)]
                   + [lambda ob=ob: ffn2(1, ob) for ob in range(DCH)]
                   + [lambda: pe_warm_tail(10)]
                   + [lambda fb=fb: ffn1(2, fb) for fb in range(NFB)]
                   + [lambda: ffn_load(3)]
                   + [lambda ob=ob: ffn2(2, ob) for ob in range(DCH)]
                   + [lambda: pe_warm_tail(20)]
                   + [lambda fb=fb: ffn1(3, fb) for fb in range(NFB)]
                   + [lambda ob=ob: ffn2(3, ob) for ob in range(DCH)]):
            th()


_CACHED_NC = None


def _build():
    global _CACHED_NC
    if _CACHED_NC is None:
        nc = bacc.Bacc(
            "TRN2",
            target_bir_lowering=False,
            debug=False,
            num_devices=N_CORES,
        )
        with tile.TileContext(nc) as tc:
            decoder_kernel(tc)
        nc.compile()
        _CACHED_NC = nc
    return _CACHED_NC


BF = ml_dtypes.bfloat16


def _pack_w(wT):
    """[1024, m] -> [128, 8*m]: d-chunk blocks side by side, partition-major."""
    m = wT.shape[1]
    return np.ascontiguousarray(
        wT.reshape(8, 128, m).transpose(1, 0, 2).reshape(128, 8 * m)
    ).astype(BF)


def _pack_x(xT, nch):
    """[1024, nch*512] -> [128, nch * 8 * 512]: per seq-chunk c, the 8
    feature-blocks of that chunk's columns, contiguous."""
    return np.ascontiguousarray(
        xT.reshape(8, 128, nch, 512).transpose(1, 2, 0, 3)
        .reshape(128, nch * 8 * 512)).astype(BF)


def make_in_maps(inputs):
    """Host-side prep: transposes + per-core weight slices + prepack (bf16)."""
    f = np.ascontiguousarray
    wembT = _pack_x(np.asarray(inputs["wemb"], np.float32).T, NSQ)
    pembT = _pack_x(np.asarray(inputs["pemb"], np.float32).T, 2)
    in_maps = []
    for i in range(N_CORES):
        hsl = slice(HD * i, HD * (i + 1))
        fsl = slice(FF_SH * i, FF_SH * (i + 1))
        w2T = np.asarray(inputs["W2"], np.float32)[:, fsl].T  # [512, 1024]
        w2h = f(w2T.reshape(4, 128, 8, 128).transpose(1, 2, 0, 3)
                .reshape(128, 4096)).astype(BF)
        in_maps.append({
            "wembT": wembT,
            "pembT": pembT,
            "wqmT": _pack_w(np.asarray(inputs["Wq_m"], np.float32)[hsl, :].T),
            "wkmT": _pack_w(np.asarray(inputs["Wk_m"], np.float32)[hsl, :].T),
            "wvmT": _pack_w(np.asarray(inputs["Wv_m"], np.float32)[hsl, :].T),
            "wqcT": _pack_w(np.asarray(inputs["Wq_c"], np.float32)[hsl, :].T),
            "wkcT": _pack_w(np.asarray(inputs["Wk_c"], np.float32)[hsl, :].T),
            "wvcT": _pack_w(np.asarray(inputs["Wv_c"], np.float32)[hsl, :].T),
            "w1T": _pack_w(np.asarray(inputs["W1"], np.float32)[fsl, :].T),
            "w2T": w2h,
        })
    return in_maps


def kernel(**inputs) -> np.ndarray:
    nc = _build()
    in_maps = make_in_maps(inputs)
    res = run_bass_kernel_spmd(nc, in_maps, core_ids=list(range(N_CORES)))
    acc = np.zeros((D_MODEL, S_W), dtype=np.float64)
    for i in range(N_CORES):
        acc += np.asarray(res.results[i]["outT"], dtype=np.float64)
    return np.ascontiguousarray(acc.T.astype(np.float32))
